# revision 8
# baseline (speedup 1.0000x reference)
"""LSH cosine-of-Hamming retrieval kernel for 8 trn2 NeuronCores.

Math: reference computes cos((pi/d) * hamming(u, v)) for binary LSH codes
u = (emb1 @ r.T > 0), v = (emb2 @ r.T > 0), d = 1024 bits.
With +/-1 sign codes s_u = 2u-1, s_v = 2v-1:
    hamming = (d - s_u . s_v) / 2
    cos((pi/d) * hamming) = cos(pi/2 - (pi/2d) * s_u.s_v) = sin((pi/2d) * s_u.s_v)

Projection: ONE fp16 matmul pass (fp16 has 11 mantissa bits; measured
sign-flip rate 8e-5 -> whole-pipeline rel_err 9.7e-3 vs the 2e-2 gate,
deterministic since inputs/reference are fixed). This is 3x cheaper than
the previous 3-pass bf16 hi/lo scheme.

Binarize: the projection PSUM must be turned into fp8 codes.  Only DVE
(vector) and ACT (scalar) can read PSUM (GPSIMD cannot).  DVE emits
+/-0.5 codes via tensor_scalar(is_gt, subtract 0.5); ACT emits +/-1
codes via the Sign activation.  Bit-chunks are assigned to engines with
complementary parity between u and v, so every bit's code product is
+/-0.5 and the final Sin just uses a 2x scale: out = sin((pi/d) * dot).

Main matmul: fp8 DoubleRow (integer-exact in PSUM f32), 216ns/MM
measured = fp8 peak. Sin activation emits bf16 (adds ~1e-3 rel err,
halves output DMA bytes); host converts back to f32.

Sharding (2x4 grid over 8 cores): core k computes the [2048, 2048] output
block for emb1 rows [(k//4)*2048...] x emb2 rows [(k%4)*2048...]; r is
replicated (collectives measure ~60us fixed cost under this runtime, so
code-sharing across cores does not pay).

Schedule (from ntff trace analysis):
- All input DMAs up front on the two HWDGE queues (sync: first-needed,
  scalar: rest, in consumption order). No gpsimd SWDGE DMAs - their exit
  dge_drain alone cost 4.2us in the baseline.
- v (e2) is projected first since the main loop needs ALL v codes but
  only the m-th row-chunk of u codes; u chunks are interleaved between
  early main m-blocks so binarize throughput (the pacing resource during
  projection: ~0.29us/tile two-engine aggregate vs 0.21us/tile PE) never
  idles the PE for long.
- No warmup matmuls: the framework prologue blocks the PE queue until
  ~7us and input data lands before that, so warmup only delays real work.
"""

import sys

sys.path.insert(0, "/opt/trn_rl_repo")

import ml_dtypes
import numpy as np

import concourse.bacc as bacc
import concourse.tile as tile
from concourse import mybir
from concourse.bass_utils import run_bass_kernel_spmd

N1, N2, D, B = 4096, 8192, 128, 1024  # emb1 rows, emb2 rows, dim, num_bits
G1, G2 = 2, 4
M1, M2 = N1 // G1, N2 // G2  # 2048 x 2048 output block per core
KC = B // 128  # 8 bit-chunks of 128
RW = 512  # projection row-chunk width
NW = 512  # main matmul output tile width

_BUILD_CACHE = {}


def _dedupe_ldweights(nc):
    """Drop back-to-back InstLdweights with identical operands on the PE
    queue. The pipeline emits one weight load per matmul; when consecutive
    matmuls share a stationary operand, the reload is pure overhead. Only
    loads carrying no semaphore waits/updates are removed, so sync
    arithmetic is unchanged; the paired matmuls then use the weights the
    earlier identical load left in the array."""
    removed = 0
    for f in nc.m.functions:
        for bb in f.blocks:
            last_key = None
            for ins in list(bb.instructions):
                if type(ins).__name__ == "InstLdweights":
                    key = ins.concise()
                    if (
                        key == last_key
                        and not ins.has_wait()
                        and not ins.has_update()
                    ):
                        bb.instructions.remove(ins)
                        removed += 1
                    else:
                        last_key = key
    return removed


def _build(scale: float):
    if scale in _BUILD_CACHE:
        return _BUILD_CACHE[scale]
    nc = bacc.Bacc("TRN2", target_bir_lowering=False, debug=False)
    f32 = mybir.dt.float32
    f16 = mybir.dt.float16
    bf16 = mybir.dt.bfloat16
    fp8 = mybir.dt.float8e4

    e1d = nc.declare_dram_parameter("e1", [D, M1], f16, isOutput=False)
    e2d = nc.declare_dram_parameter("e2", [D, M2], f16, isOutput=False)
    rd = nc.declare_dram_parameter("r", [D, B], f16, isOutput=False)
    out = nc.declare_dram_parameter("out", [M1, M2], bf16, isOutput=True)
    junkd = nc.declare_dram_parameter("junk", [128, 8], bf16, isOutput=True)

    with tile.TileContext(nc) as tc:
        with (
            tc.tile_pool(name="const", bufs=1) as const_pool,
            tc.tile_pool(name="codes", bufs=1) as code_pool,
            tc.tile_pool(name="outs", bufs=4) as out_pool,
            tc.tile_pool(name="psum", bufs=4, space="PSUM") as psum_pool,
        ):
            r_sb = const_pool.tile([D, B], f16)
            e1_sb = const_pool.tile([D, M1], f16)
            e2_sb = const_pool.tile([D, M2], f16)

            ut = code_pool.tile([128, KC, M1], fp8)
            vt = code_pool.tile([128, KC, M2], fp8)

            junk_sb = out_pool.tile([128, 8], bf16)

            # Input DMAs up front on the two HWDGE queues, few and fat
            # (wide rows = big descriptors), first-needed pieces first:
            # sync carries the tiny r head + first quarter-chunk of e2 the
            # first matmuls need; scalar streams everything else in
            # consumption order.
            nc.sync.dma_start(r_sb[:, 0:128], rd[:, 0:128])
            nc.sync.dma_start(e2_sb[:, 0:256], e2d[:, 0:256])
            nc.scalar.dma_start(r_sb[:, 128:], rd[:, 128:])
            nc.scalar.dma_start(e2_sb[:, 256:512], e2d[:, 256:512])
            nc.scalar.dma_start(e2_sb[:, 512:1024], e2d[:, 512:1024])
            nc.scalar.dma_start(e2_sb[:, 1024:2048], e2d[:, 1024:2048])
            nc.scalar.dma_start(e1_sb[:, 0:1024], e1d[:, 0:1024])
            nc.scalar.dma_start(e1_sb[:, 1024:2048], e1d[:, 1024:2048])

            # Warmup: the PE queue clears its prologue ~2.5us before the
            # first input DMA lands; garbage matmuls in that window keep
            # the PE busy so the HAM clock-gate hits 2.4 GHz right as real
            # work starts. The memset runs on gpsimd (idle early, so the
            # matmuls wait on nothing slow); a tiny consumer DMA'd to a
            # junk output keeps DCE from dropping them.
            warm = const_pool.tile([128, RW], f16)
            nc.gpsimd.memset(warm[:], 0.0)
            wps = psum_pool.tile([128, 2, RW], f32, name="pstile", tag="ps")
            for w in range(5):
                nc.tensor.matmul(
                    wps[:, w % 2, :], warm[:, 0:128], warm[:],
                    start=(w < 2), stop=(w >= 3),
                )
            nc.vector.tensor_scalar(
                junk_sb[:], wps[:, 0, 0:8], 0.0, 0.5,
                mybir.AluOpType.is_gt, mybir.AluOpType.subtract,
            )
            nc.sync.dma_start(junkd[:, :], junk_sb[:])

            def proj_chunk(esb, dst, j, act_parity, w=RW, half=0):
                """Project a row-chunk of one side and binarize to fp8
                codes. Bit-chunk-pairs with c2 % 2 == act_parity go to ACT
                (Sign, +/-1 codes), the rest to DVE (is_gt-0.5, +/-0.5).
                u and v use opposite parity so every bit's u*v code
                product is +/-0.5."""
                sl = slice(j * RW + half * w, j * RW + (half + 1) * w)
                for c2 in range(KC // 2):
                    ps = psum_pool.tile([128, 2, w], f32, name="pstile", tag="ps")
                    for h in range(2):
                        cs = slice((2 * c2 + h) * 128, (2 * c2 + h + 1) * 128)
                        nc.tensor.matmul(
                            ps[:, h, :], r_sb[:, cs], esb[:, sl],
                            start=True, stop=True,
                        )
                    if c2 % 2 == act_parity:
                        nc.scalar.activation(
                            dst[:, 2 * c2 : 2 * c2 + 2, sl],
                            ps[:],
                            mybir.ActivationFunctionType.Sign,
                        )
                    else:
                        nc.vector.tensor_scalar(
                            dst[:, 2 * c2 : 2 * c2 + 2, sl],
                            ps[:],
                            0.0,
                            0.5,
                            mybir.AluOpType.is_gt,
                            mybir.AluOpType.subtract,
                        )

            def main_block(m):
                """One 128-row output block: fp8 DoubleRow matmul over all
                1024 bits, Sin to bf16, one 512KB output DMA (alternating
                between the two HWDGE queues so neither backs up)."""
                ot = out_pool.tile([128, M2], bf16)
                ms = slice(m * 128, (m + 1) * 128)
                pss = [
                    psum_pool.tile([128, 2, NW], f32, name="pstile", tag="ps")
                    for _ in range(2)
                ]
                for s in range(KC // 2):
                    for t in range(2):
                        for h in range(2):
                            n = 2 * t + h
                            ns = slice(n * NW, (n + 1) * NW)
                            nc.tensor.matmul(
                                pss[t][:, h, :],
                                ut[:, 2 * s : 2 * s + 2, ms],
                                vt[:, 2 * s : 2 * s + 2, ns],
                                start=(s == 0),
                                stop=(s == KC // 2 - 1),
                                perf_mode=mybir.MatmulPerfMode.DoubleRow,
                            )
                for t in range(2):
                    os_ = slice(t * 2 * NW, (t + 1) * 2 * NW)
                    nc.scalar.activation(
                        ot[:, os_],
                        pss[t][:],
                        mybir.ActivationFunctionType.Sin,
                        scale=scale,
                    )
                (nc.sync if m % 2 == 0 else nc.scalar).dma_start(out[ms, :], ot[:])

            # v first (ACT takes odd bit-pairs), then u chunks (ACT takes
            # even) interleaved with the first main blocks so binarize
            # never stalls the PE: m-block m needs u-chunk m//4 and ALL v.
            # v-chunk 0 runs as two 256-wide halves so its first matmuls
            # only need the small head DMAs.
            proj_chunk(e2_sb, vt, 0, act_parity=1, w=256, half=0)
            proj_chunk(e2_sb, vt, 0, act_parity=1, w=256, half=1)
            for j in range(1, M2 // RW):
                proj_chunk(e2_sb, vt, j, act_parity=1)
            proj_chunk(e1_sb, ut, 0, act_parity=0)
            main_block(0)
            proj_chunk(e1_sb, ut, 1, act_parity=0)
            main_block(1)
            proj_chunk(e1_sb, ut, 2, act_parity=0)
            main_block(2)
            main_block(3)
            proj_chunk(e1_sb, ut, 3, act_parity=0)
            for m in range(4, M1 // 128):
                main_block(m)

    # Keep waits on the matmuls (not hoisted to ldweights) so redundant
    # weight loads stay sync-free and can be deduped away.
    nc.move_matmul_waits_to_ldweights = lambda: None
    nc.compile()
    _dedupe_ldweights(nc)
    _BUILD_CACHE[scale] = nc
    return nc


def _in_maps(emb1, emb2, r):
    r16 = np.ascontiguousarray(r.T).astype(np.float16)
    e1t = np.ascontiguousarray(emb1.T).astype(np.float16)
    e2t = np.ascontiguousarray(emb2.T).astype(np.float16)
    maps = []
    for k in range(8):
        a, b = k // G2, k % G2
        s1 = slice(a * M1, (a + 1) * M1)
        s2 = slice(b * M2, (b + 1) * M2)
        maps.append(
            {
                "e1": np.ascontiguousarray(e1t[:, s1]),
                "e2": np.ascontiguousarray(e2t[:, s2]),
                "r": r16,
            }
        )
    return maps


def _install_profile_hook():
    """The agent image's antenv lacks axon_hooks; synthesize it so
    run_bass_kernel_spmd(trace=True) can reach the NTFF profiler."""
    import types

    if "antenv.axon_hooks" in sys.modules:
        return
    try:
        from trn_agent_boot.trn_boot import _ntff_profile_via_ctypes

        hook = _ntff_profile_via_ctypes("/opt/axon/libaxon_pjrt.so")
        mod = types.ModuleType("antenv.axon_hooks")
        mod.get_axon_ntff_profile_hook = lambda: hook
        sys.modules["antenv.axon_hooks"] = mod

        from concourse import bass_utils as _bu

        _orig_upload = _bu.upload_artifacts

        def _safe_upload(tmpdir):
            try:
                return _orig_upload(tmpdir)
            except Exception as e:  # no bucket access in this container
                return f"upload-skipped: {e}"

        _bu.upload_artifacts = _safe_upload
    except Exception:
        pass


def kernel(emb1, emb2, r, pi, _trace=False, _tmpdir=None):
    emb1 = np.asarray(emb1, dtype=np.float32)
    emb2 = np.asarray(emb2, dtype=np.float32)
    r = np.asarray(r, dtype=np.float32)
    # per-bit code product is +/-0.5 -> dot = s_u.s_v / 2, out = sin((pi/d)*dot)
    scale = float(np.asarray(pi).reshape(-1)[0]) / B

    nc = _build(scale)
    if _trace:
        _install_profile_hook()
    try:
        res = run_bass_kernel_spmd(
            nc, _in_maps(emb1, emb2, r), list(range(8)), trace=_trace, tmpdir=_tmpdir
        )
    except ModuleNotFoundError:
        res = run_bass_kernel_spmd(nc, _in_maps(emb1, emb2, r), list(range(8)))

    full = np.empty((N1, N2), dtype=np.float32)
    for k in range(8):
        a, b = k // G2, k % G2
        blk = np.asarray(res.results[k]["out"]).astype(np.float32)
        full[a * M1 : (a + 1) * M1, b * M2 : (b + 1) * M2] = blk
    if _trace:
        kernel._last_exec_time_ns = res.exec_time_ns
    return full


# revision 11
# speedup vs baseline: 1.1599x; 1.1599x over previous
"""LSH cosine-of-Hamming retrieval kernel for 8 trn2 NeuronCores.

Math: reference computes cos((pi/d) * hamming(u, v)) for binary LSH codes
u = (emb1 @ r.T > 0), v = (emb2 @ r.T > 0), d = 1024 bits.
With +/-1 sign codes s_u = 2u-1, s_v = 2v-1:
    hamming = (d - s_u . s_v) / 2
    cos((pi/d) * hamming) = cos(pi/2 - (pi/2d) * s_u.s_v) = sin((pi/2d) * s_u.s_v)

Projection: ONE fp16 matmul pass (fp16 has 11 mantissa bits; measured
sign-flip rate 8e-5 -> whole-pipeline rel_err 9.7e-3 vs the 2e-2 gate,
deterministic since inputs/reference are fixed). This is 3x cheaper than
the previous 3-pass bf16 hi/lo scheme.

Binarize: the projection PSUM must be turned into fp8 codes.  Only DVE
(vector) and ACT (scalar) can read PSUM (GPSIMD cannot).  DVE emits
+/-0.5 codes via tensor_scalar(is_gt, subtract 0.5); ACT emits +/-1
codes via the Sign activation.  Bit-chunks are assigned to engines with
complementary parity between u and v, so every bit's code product is
+/-0.5 and the final Sin just uses a 2x scale: out = sin((pi/d) * dot).

Main matmul: fp8 DoubleRow (integer-exact in PSUM f32), 216ns/MM
measured = fp8 peak. Sin activation emits bf16 (adds ~1e-3 rel err,
halves output DMA bytes); host converts back to f32.

Sharding (2x4 grid over 8 cores): core k computes the [2048, 2048] output
block for emb1 rows [(k//4)*2048...] x emb2 rows [(k%4)*2048...]; r is
replicated (collectives measure ~60us fixed cost under this runtime, so
code-sharing across cores does not pay).

Schedule (from ntff trace analysis):
- All input DMAs up front on the two HWDGE queues (sync: first-needed,
  scalar: rest, in consumption order). No gpsimd SWDGE DMAs - their exit
  dge_drain alone cost 4.2us in the baseline.
- v (e2) is projected first since the main loop needs ALL v codes but
  only the m-th row-chunk of u codes; u chunks are interleaved between
  early main m-blocks so binarize throughput (the pacing resource during
  projection: ~0.29us/tile two-engine aggregate vs 0.21us/tile PE) never
  idles the PE for long.
- No warmup matmuls: the framework prologue blocks the PE queue until
  ~7us and input data lands before that, so warmup only delays real work.
"""

import sys

sys.path.insert(0, "/opt/trn_rl_repo")

import ml_dtypes
import numpy as np

import concourse.bacc as bacc
import concourse.tile as tile
from concourse import mybir
from concourse.bass_utils import run_bass_kernel_spmd

N1, N2, D, B = 4096, 8192, 128, 1024  # emb1 rows, emb2 rows, dim, num_bits
G1, G2 = 2, 4
M1, M2 = N1 // G1, N2 // G2  # 2048 x 2048 output block per core
KC = B // 128  # 8 bit-chunks of 128
RW = 512  # projection row-chunk width
NW = 512  # main matmul output tile width

_BUILD_CACHE = {}


def _dedupe_ldweights(nc):
    """Drop back-to-back InstLdweights with identical operands on the PE
    queue. The pipeline emits one weight load per matmul; when consecutive
    matmuls share a stationary operand, the reload is pure overhead. Only
    loads carrying no semaphore waits/updates are removed, so sync
    arithmetic is unchanged; the paired matmuls then use the weights the
    earlier identical load left in the array."""
    removed = 0
    for f in nc.m.functions:
        for bb in f.blocks:
            last_key = None
            for ins in list(bb.instructions):
                if type(ins).__name__ == "InstLdweights":
                    key = ins.concise()
                    if (
                        key == last_key
                        and not ins.has_wait()
                        and not ins.has_update()
                    ):
                        bb.instructions.remove(ins)
                        removed += 1
                    else:
                        last_key = key
    return removed


def _build(scale: float):
    if scale in _BUILD_CACHE:
        return _BUILD_CACHE[scale]
    nc = bacc.Bacc("TRN2", target_bir_lowering=False, debug=False)
    f32 = mybir.dt.float32
    f16 = mybir.dt.float16
    bf16 = mybir.dt.bfloat16
    fp8 = mybir.dt.float8e4

    e1d = nc.declare_dram_parameter("e1", [D, M1], f16, isOutput=False)
    e2d = nc.declare_dram_parameter("e2", [D, M2], f16, isOutput=False)
    rd = nc.declare_dram_parameter("r", [D, B], f16, isOutput=False)
    out = nc.declare_dram_parameter("out", [M1, M2], bf16, isOutput=True)
    junkd = nc.declare_dram_parameter("junk", [128, 8], bf16, isOutput=True)

    with tile.TileContext(nc) as tc:
        with (
            tc.tile_pool(name="const", bufs=1) as const_pool,
            tc.tile_pool(name="codes", bufs=1) as code_pool,
            tc.tile_pool(name="outs", bufs=4) as out_pool,
            tc.tile_pool(name="psum", bufs=4, space="PSUM") as psum_pool,
        ):
            r_sb = const_pool.tile([D, B], f16)
            e1_sb = const_pool.tile([D, M1], f16)
            e2_sb = const_pool.tile([D, M2], f16)

            ut = code_pool.tile([128, KC, M1], fp8)
            vt = code_pool.tile([128, KC, M2], fp8)

            junk_sb = const_pool.tile([128, 8], bf16)

            # Input DMAs up front on the two HWDGE queues, few and fat
            # (wide rows = big descriptors), first-needed pieces first:
            # sync carries the tiny r head + first quarter-chunk of e2 the
            # first matmuls need; scalar streams everything else in
            # consumption order.
            nc.sync.dma_start(r_sb[:, 0:128], rd[:, 0:128])
            nc.sync.dma_start(e2_sb[:, 0:256], e2d[:, 0:256])
            nc.scalar.dma_start(r_sb[:, 128:], rd[:, 128:])
            nc.scalar.dma_start(e2_sb[:, 256:512], e2d[:, 256:512])
            nc.scalar.dma_start(e2_sb[:, 512:1024], e2d[:, 512:1024])
            nc.scalar.dma_start(e2_sb[:, 1024:2048], e2d[:, 1024:2048])
            nc.scalar.dma_start(e1_sb[:, 0:1024], e1d[:, 0:1024])
            nc.scalar.dma_start(e1_sb[:, 1024:2048], e1d[:, 1024:2048])

            # Warmup: the PE queue clears its prologue ~2.5us before the
            # first input DMA lands; garbage matmuls in that window keep
            # the PE busy so the HAM clock-gate hits 2.4 GHz right as real
            # work starts. The memset runs on gpsimd (idle early, so the
            # matmuls wait on nothing slow); a tiny consumer DMA'd to a
            # junk output keeps DCE from dropping them.
            warm = const_pool.tile([128, RW], f16)
            nc.vector.memset(warm[:], 0.0)
            wps = psum_pool.tile([128, 2, RW], f32, name="pstile", tag="ps")
            for w in range(5):
                nc.tensor.matmul(
                    wps[:, w % 2, :], warm[:, 0:128], warm[:],
                    start=(w < 2), stop=(w >= 3),
                )
            nc.vector.tensor_scalar(
                junk_sb[:], wps[:, 0, 0:8], 0.0, 0.5,
                mybir.AluOpType.is_gt, mybir.AluOpType.subtract,
            )
            nc.sync.dma_start(junkd[:, :], junk_sb[:])

            def proj_chunk(esb, dst, j, act_parity, w=RW, half=0):
                """Project a row-chunk of one side and binarize to fp8
                codes. Bit-chunk-pairs with c2 % 2 == act_parity go to ACT
                (Sign, +/-1 codes), the rest to DVE (is_gt-0.5, +/-0.5).
                u and v use opposite parity so every bit's u*v code
                product is +/-0.5."""
                sl = slice(j * RW + half * w, j * RW + (half + 1) * w)
                for c2 in range(KC // 2):
                    ps = psum_pool.tile([128, 2, w], f32, name="pstile", tag="ps")
                    for h in range(2):
                        cs = slice((2 * c2 + h) * 128, (2 * c2 + h + 1) * 128)
                        nc.tensor.matmul(
                            ps[:, h, :], r_sb[:, cs], esb[:, sl],
                            start=True, stop=True,
                        )
                    if c2 % 2 == act_parity:
                        nc.scalar.activation(
                            dst[:, 2 * c2 : 2 * c2 + 2, sl],
                            ps[:],
                            mybir.ActivationFunctionType.Sign,
                        )
                    else:
                        nc.vector.tensor_scalar(
                            dst[:, 2 * c2 : 2 * c2 + 2, sl],
                            ps[:],
                            0.0,
                            0.5,
                            mybir.AluOpType.is_gt,
                            mybir.AluOpType.subtract,
                        )

            def main_block(m):
                """One 128-row output block: fp8 DoubleRow matmul over all
                1024 bits, Sin to bf16, one 512KB output DMA (alternating
                between the two HWDGE queues so neither backs up)."""
                ot = out_pool.tile([128, M2], bf16)
                ms = slice(m * 128, (m + 1) * 128)
                pss = [
                    psum_pool.tile([128, 2, NW], f32, name="pstile", tag="ps")
                    for _ in range(2)
                ]
                for s in range(KC // 2):
                    for t in range(2):
                        for h in range(2):
                            n = 2 * t + h
                            ns = slice(n * NW, (n + 1) * NW)
                            nc.tensor.matmul(
                                pss[t][:, h, :],
                                ut[:, 2 * s : 2 * s + 2, ms],
                                vt[:, 2 * s : 2 * s + 2, ns],
                                start=(s == 0),
                                stop=(s == KC // 2 - 1),
                                perf_mode=mybir.MatmulPerfMode.DoubleRow,
                            )
                for t in range(2):
                    os_ = slice(t * 2 * NW, (t + 1) * 2 * NW)
                    nc.scalar.activation(
                        ot[:, os_],
                        pss[t][:],
                        mybir.ActivationFunctionType.Sin,
                        scale=scale,
                    )
                    # store each half as soon as its Sin lands; sync owns
                    # all output traffic (its 16-channel ring set has
                    # plenty of headroom) so the ACT queue stays clean
                    nc.sync.dma_start(out[ms, os_], ot[:, os_])

            # v first (ACT takes odd bit-pairs), then u chunks (ACT takes
            # even) interleaved with the first main blocks so binarize
            # never stalls the PE: m-block m needs u-chunk m//4 and ALL v.
            # v-chunk 0 runs as two 256-wide halves so its first matmuls
            # only need the small head DMAs.
            proj_chunk(e2_sb, vt, 0, act_parity=1, w=256, half=0)
            proj_chunk(e2_sb, vt, 0, act_parity=1, w=256, half=1)
            for j in range(1, M2 // RW):
                proj_chunk(e2_sb, vt, j, act_parity=1)
            proj_chunk(e1_sb, ut, 0, act_parity=0)
            main_block(0)
            proj_chunk(e1_sb, ut, 1, act_parity=0)
            main_block(1)
            proj_chunk(e1_sb, ut, 2, act_parity=0)
            main_block(2)
            main_block(3)
            proj_chunk(e1_sb, ut, 3, act_parity=0)
            for m in range(4, M1 // 128):
                main_block(m)

    # Keep waits on the matmuls (not hoisted to ldweights) so redundant
    # weight loads stay sync-free and can be deduped away.
    nc.move_matmul_waits_to_ldweights = lambda: None
    nc.compile()
    _dedupe_ldweights(nc)
    _BUILD_CACHE[scale] = nc
    return nc


def _in_maps(emb1, emb2, r):
    r16 = np.ascontiguousarray(r.T).astype(np.float16)
    e1t = np.ascontiguousarray(emb1.T).astype(np.float16)
    e2t = np.ascontiguousarray(emb2.T).astype(np.float16)
    maps = []
    for k in range(8):
        a, b = k // G2, k % G2
        s1 = slice(a * M1, (a + 1) * M1)
        s2 = slice(b * M2, (b + 1) * M2)
        maps.append(
            {
                "e1": np.ascontiguousarray(e1t[:, s1]),
                "e2": np.ascontiguousarray(e2t[:, s2]),
                "r": r16,
            }
        )
    return maps


def _install_profile_hook():
    """The agent image's antenv lacks axon_hooks; synthesize it so
    run_bass_kernel_spmd(trace=True) can reach the NTFF profiler."""
    import types

    if "antenv.axon_hooks" in sys.modules:
        return
    try:
        from trn_agent_boot.trn_boot import _ntff_profile_via_ctypes

        hook = _ntff_profile_via_ctypes("/opt/axon/libaxon_pjrt.so")
        mod = types.ModuleType("antenv.axon_hooks")
        mod.get_axon_ntff_profile_hook = lambda: hook
        sys.modules["antenv.axon_hooks"] = mod

        from concourse import bass_utils as _bu

        _orig_upload = _bu.upload_artifacts

        def _safe_upload(tmpdir):
            try:
                return _orig_upload(tmpdir)
            except Exception as e:  # no bucket access in this container
                return f"upload-skipped: {e}"

        _bu.upload_artifacts = _safe_upload
    except Exception:
        pass


def kernel(emb1, emb2, r, pi, _trace=False, _tmpdir=None):
    emb1 = np.asarray(emb1, dtype=np.float32)
    emb2 = np.asarray(emb2, dtype=np.float32)
    r = np.asarray(r, dtype=np.float32)
    # per-bit code product is +/-0.5 -> dot = s_u.s_v / 2, out = sin((pi/d)*dot)
    scale = float(np.asarray(pi).reshape(-1)[0]) / B

    nc = _build(scale)
    if _trace:
        _install_profile_hook()
    try:
        res = run_bass_kernel_spmd(
            nc, _in_maps(emb1, emb2, r), list(range(8)), trace=_trace, tmpdir=_tmpdir
        )
    except ModuleNotFoundError:
        res = run_bass_kernel_spmd(nc, _in_maps(emb1, emb2, r), list(range(8)))

    full = np.empty((N1, N2), dtype=np.float32)
    for k in range(8):
        a, b = k // G2, k % G2
        blk = np.asarray(res.results[k]["out"]).astype(np.float32)
        full[a * M1 : (a + 1) * M1, b * M2 : (b + 1) * M2] = blk
    if _trace:
        kernel._last_exec_time_ns = res.exec_time_ns
    return full


# revision 12
# speedup vs baseline: 1.1874x; 1.0237x over previous
"""LSH cosine-of-Hamming retrieval kernel for 8 trn2 NeuronCores.

Math: reference computes cos((pi/d) * hamming(u, v)) for binary LSH codes
u = (emb1 @ r.T > 0), v = (emb2 @ r.T > 0), d = 1024 bits.
With +/-1 sign codes s_u = 2u-1, s_v = 2v-1:
    hamming = (d - s_u . s_v) / 2
    cos((pi/d) * hamming) = cos(pi/2 - (pi/2d) * s_u.s_v) = sin((pi/2d) * s_u.s_v)

Projection: ONE fp16 matmul pass (fp16 has 11 mantissa bits; measured
sign-flip rate 8e-5 -> whole-pipeline rel_err 9.7e-3 vs the 2e-2 gate,
deterministic since inputs/reference are fixed). This is 3x cheaper than
the previous 3-pass bf16 hi/lo scheme.

Binarize: the projection PSUM must be turned into fp8 codes.  Only DVE
(vector) and ACT (scalar) can read PSUM (GPSIMD cannot).  DVE emits
+/-0.5 codes via tensor_scalar(is_gt, subtract 0.5); ACT emits +/-1
codes via the Sign activation.  Bit-chunks are assigned to engines with
complementary parity between u and v, so every bit's code product is
+/-0.5 and the final Sin just uses a 2x scale: out = sin((pi/d) * dot).

Main matmul: fp8 DoubleRow (integer-exact in PSUM f32), 216ns/MM
measured = fp8 peak. Sin activation emits bf16 (adds ~1e-3 rel err,
halves output DMA bytes); host converts back to f32.

Sharding (2x4 grid over 8 cores): core k computes the [2048, 2048] output
block for emb1 rows [(k//4)*2048...] x emb2 rows [(k%4)*2048...]; r is
replicated (collectives measure ~60us fixed cost under this runtime, so
code-sharing across cores does not pay).

Schedule (from ntff trace analysis):
- All input DMAs up front on the two HWDGE queues (sync: first-needed,
  scalar: rest, in consumption order). No gpsimd SWDGE DMAs - their exit
  dge_drain alone cost 4.2us in the baseline.
- v (e2) is projected first since the main loop needs ALL v codes but
  only the m-th row-chunk of u codes; u chunks are interleaved between
  early main m-blocks so binarize throughput (the pacing resource during
  projection: ~0.29us/tile two-engine aggregate vs 0.21us/tile PE) never
  idles the PE for long.
- No warmup matmuls: the framework prologue blocks the PE queue until
  ~7us and input data lands before that, so warmup only delays real work.
"""

import sys

sys.path.insert(0, "/opt/trn_rl_repo")

import ml_dtypes
import numpy as np

import concourse.bacc as bacc
import concourse.tile as tile
from concourse import mybir
from concourse.bass_utils import run_bass_kernel_spmd

N1, N2, D, B = 4096, 8192, 128, 1024  # emb1 rows, emb2 rows, dim, num_bits
G1, G2 = 2, 4
M1, M2 = N1 // G1, N2 // G2  # 2048 x 2048 output block per core
KC = B // 128  # 8 bit-chunks of 128
RW = 512  # projection row-chunk width
NW = 512  # main matmul output tile width

_BUILD_CACHE = {}


def _dedupe_ldweights(nc):
    """Drop back-to-back InstLdweights with identical operands on the PE
    queue. The pipeline emits one weight load per matmul; when consecutive
    matmuls share a stationary operand, the reload is pure overhead. Only
    loads carrying no semaphore waits/updates are removed, so sync
    arithmetic is unchanged; the paired matmuls then use the weights the
    earlier identical load left in the array."""
    removed = 0
    for f in nc.m.functions:
        for bb in f.blocks:
            last_key = None
            for ins in list(bb.instructions):
                if type(ins).__name__ == "InstLdweights":
                    key = ins.concise()
                    if (
                        key == last_key
                        and not ins.has_wait()
                        and not ins.has_update()
                    ):
                        bb.instructions.remove(ins)
                        removed += 1
                    else:
                        last_key = key
    return removed


def _build(scale: float):
    if scale in _BUILD_CACHE:
        return _BUILD_CACHE[scale]
    nc = bacc.Bacc("TRN2", target_bir_lowering=False, debug=False)
    f32 = mybir.dt.float32
    f16 = mybir.dt.float16
    bf16 = mybir.dt.bfloat16
    fp8 = mybir.dt.float8e4

    e1d = nc.declare_dram_parameter("e1", [D, M1], f16, isOutput=False)
    e2d = nc.declare_dram_parameter("e2", [D, M2], f16, isOutput=False)
    rd = nc.declare_dram_parameter("r", [D, B], f16, isOutput=False)
    out = nc.declare_dram_parameter("out", [M1, M2], bf16, isOutput=True)
    junkd = nc.declare_dram_parameter("junk", [128, 8], bf16, isOutput=True)

    with tile.TileContext(nc) as tc:
        with (
            tc.tile_pool(name="const", bufs=1) as const_pool,
            tc.tile_pool(name="codes", bufs=1) as code_pool,
            tc.tile_pool(name="outs", bufs=4) as out_pool,
            tc.tile_pool(name="psum", bufs=4, space="PSUM") as psum_pool,
        ):
            r_sb = const_pool.tile([D, B], f16)
            e1_sb = const_pool.tile([D, M1], f16)
            e2_sb = const_pool.tile([D, M2], f16)

            ut = code_pool.tile([128, KC, M1], fp8)
            vt = code_pool.tile([128, KC, M2], fp8)

            junk_sb = const_pool.tile([128, 8], bf16)

            # Input DMAs up front on the two HWDGE queues, few and fat
            # (wide rows = big descriptors), first-needed pieces first:
            # sync carries the tiny r head + first quarter-chunk of e2 the
            # first matmuls need; scalar streams everything else in
            # consumption order.
            nc.sync.dma_start(r_sb[:, 0:128], rd[:, 0:128])
            nc.sync.dma_start(e2_sb[:, 0:256], e2d[:, 0:256])
            nc.scalar.dma_start(r_sb[:, 128:], rd[:, 128:])
            nc.scalar.dma_start(e2_sb[:, 256:512], e2d[:, 256:512])
            nc.scalar.dma_start(e2_sb[:, 512:1024], e2d[:, 512:1024])
            nc.scalar.dma_start(e2_sb[:, 1024:2048], e2d[:, 1024:2048])
            nc.scalar.dma_start(e1_sb[:, 0:1024], e1d[:, 0:1024])
            nc.scalar.dma_start(e1_sb[:, 1024:2048], e1d[:, 1024:2048])

            # Warmup: the PE queue clears its prologue ~2.5us before the
            # first input DMA lands; garbage matmuls in that window keep
            # the PE busy so the HAM clock-gate hits 2.4 GHz right as real
            # work starts. The memset runs on gpsimd (idle early, so the
            # matmuls wait on nothing slow); a tiny consumer DMA'd to a
            # junk output keeps DCE from dropping them.
            warm = const_pool.tile([128, RW], f16)
            nc.vector.memset(warm[:], 0.0)
            # 9 cold matmuls ~= 3.8us: long enough to reach the first input
            # DMA's landing (~10.9us) with no PE gap, so the HAM activity
            # window stays unbroken and fires right as real work starts.
            wps = psum_pool.tile([128, 2, RW], f32, name="pstile", tag="ps")
            for w in range(9):
                nc.tensor.matmul(
                    wps[:, w % 2, :], warm[:, 0:128], warm[:],
                    start=(w < 2), stop=(w >= 7),
                )
            nc.vector.tensor_scalar(
                junk_sb[:], wps[:, 0, 0:8], 0.0, 0.5,
                mybir.AluOpType.is_gt, mybir.AluOpType.subtract,
            )
            nc.sync.dma_start(junkd[:, :], junk_sb[:])

            def proj_chunk(esb, dst, j, act_parity, w=RW, half=0):
                """Project a row-chunk of one side and binarize to fp8
                codes. Bit-chunk-pairs with c2 % 2 == act_parity go to ACT
                (Sign, +/-1 codes), the rest to DVE (is_gt-0.5, +/-0.5).
                u and v use opposite parity so every bit's u*v code
                product is +/-0.5."""
                sl = slice(j * RW + half * w, j * RW + (half + 1) * w)
                for c2 in range(KC // 2):
                    ps = psum_pool.tile([128, 2, w], f32, name="pstile", tag="ps")
                    for h in range(2):
                        cs = slice((2 * c2 + h) * 128, (2 * c2 + h + 1) * 128)
                        nc.tensor.matmul(
                            ps[:, h, :], r_sb[:, cs], esb[:, sl],
                            start=True, stop=True,
                        )
                    if c2 % 2 == act_parity:
                        nc.scalar.activation(
                            dst[:, 2 * c2 : 2 * c2 + 2, sl],
                            ps[:],
                            mybir.ActivationFunctionType.Sign,
                        )
                    else:
                        nc.vector.tensor_scalar(
                            dst[:, 2 * c2 : 2 * c2 + 2, sl],
                            ps[:],
                            0.0,
                            0.5,
                            mybir.AluOpType.is_gt,
                            mybir.AluOpType.subtract,
                        )

            def main_block(m):
                """One 128-row output block: fp8 DoubleRow matmul over all
                1024 bits, Sin to bf16, one 512KB output DMA (alternating
                between the two HWDGE queues so neither backs up)."""
                ot = out_pool.tile([128, M2], bf16)
                ms = slice(m * 128, (m + 1) * 128)
                pss = [
                    psum_pool.tile([128, 2, NW], f32, name="pstile", tag="ps")
                    for _ in range(2)
                ]
                for s in range(KC // 2):
                    for t in range(2):
                        for h in range(2):
                            n = 2 * t + h
                            ns = slice(n * NW, (n + 1) * NW)
                            nc.tensor.matmul(
                                pss[t][:, h, :],
                                ut[:, 2 * s : 2 * s + 2, ms],
                                vt[:, 2 * s : 2 * s + 2, ns],
                                start=(s == 0),
                                stop=(s == KC // 2 - 1),
                                perf_mode=mybir.MatmulPerfMode.DoubleRow,
                            )
                for t in range(2):
                    os_ = slice(t * 2 * NW, (t + 1) * 2 * NW)
                    nc.scalar.activation(
                        ot[:, os_],
                        pss[t][:],
                        mybir.ActivationFunctionType.Sin,
                        scale=scale,
                    )
                    # store each half as soon as its Sin lands; sync owns
                    # all output traffic (its 16-channel ring set has
                    # plenty of headroom) so the ACT queue stays clean
                    nc.sync.dma_start(out[ms, os_], ot[:, os_])

            # v first (ACT takes odd bit-pairs), then u chunks (ACT takes
            # even) interleaved with the first main blocks so binarize
            # never stalls the PE: m-block m needs u-chunk m//4 and ALL v.
            # v-chunk 0 runs as two 256-wide halves so its first matmuls
            # only need the small head DMAs.
            proj_chunk(e2_sb, vt, 0, act_parity=1, w=256, half=0)
            proj_chunk(e2_sb, vt, 0, act_parity=1, w=256, half=1)
            for j in range(1, M2 // RW):
                proj_chunk(e2_sb, vt, j, act_parity=1)
            proj_chunk(e1_sb, ut, 0, act_parity=0)
            main_block(0)
            proj_chunk(e1_sb, ut, 1, act_parity=0)
            main_block(1)
            proj_chunk(e1_sb, ut, 2, act_parity=0)
            main_block(2)
            main_block(3)
            proj_chunk(e1_sb, ut, 3, act_parity=0)
            for m in range(4, M1 // 128):
                main_block(m)

    # Keep waits on the matmuls (not hoisted to ldweights) so redundant
    # weight loads stay sync-free and can be deduped away.
    nc.move_matmul_waits_to_ldweights = lambda: None
    nc.compile()
    _dedupe_ldweights(nc)
    _BUILD_CACHE[scale] = nc
    return nc


def _in_maps(emb1, emb2, r):
    r16 = np.ascontiguousarray(r.T).astype(np.float16)
    e1t = np.ascontiguousarray(emb1.T).astype(np.float16)
    e2t = np.ascontiguousarray(emb2.T).astype(np.float16)
    maps = []
    for k in range(8):
        a, b = k // G2, k % G2
        s1 = slice(a * M1, (a + 1) * M1)
        s2 = slice(b * M2, (b + 1) * M2)
        maps.append(
            {
                "e1": np.ascontiguousarray(e1t[:, s1]),
                "e2": np.ascontiguousarray(e2t[:, s2]),
                "r": r16,
            }
        )
    return maps


def _install_profile_hook():
    """The agent image's antenv lacks axon_hooks; synthesize it so
    run_bass_kernel_spmd(trace=True) can reach the NTFF profiler."""
    import types

    if "antenv.axon_hooks" in sys.modules:
        return
    try:
        from trn_agent_boot.trn_boot import _ntff_profile_via_ctypes

        hook = _ntff_profile_via_ctypes("/opt/axon/libaxon_pjrt.so")
        mod = types.ModuleType("antenv.axon_hooks")
        mod.get_axon_ntff_profile_hook = lambda: hook
        sys.modules["antenv.axon_hooks"] = mod

        from concourse import bass_utils as _bu

        _orig_upload = _bu.upload_artifacts

        def _safe_upload(tmpdir):
            try:
                return _orig_upload(tmpdir)
            except Exception as e:  # no bucket access in this container
                return f"upload-skipped: {e}"

        _bu.upload_artifacts = _safe_upload
    except Exception:
        pass


def kernel(emb1, emb2, r, pi, _trace=False, _tmpdir=None):
    emb1 = np.asarray(emb1, dtype=np.float32)
    emb2 = np.asarray(emb2, dtype=np.float32)
    r = np.asarray(r, dtype=np.float32)
    # per-bit code product is +/-0.5 -> dot = s_u.s_v / 2, out = sin((pi/d)*dot)
    scale = float(np.asarray(pi).reshape(-1)[0]) / B

    nc = _build(scale)
    if _trace:
        _install_profile_hook()
    try:
        res = run_bass_kernel_spmd(
            nc, _in_maps(emb1, emb2, r), list(range(8)), trace=_trace, tmpdir=_tmpdir
        )
    except ModuleNotFoundError:
        res = run_bass_kernel_spmd(nc, _in_maps(emb1, emb2, r), list(range(8)))

    full = np.empty((N1, N2), dtype=np.float32)
    for k in range(8):
        a, b = k // G2, k % G2
        blk = np.asarray(res.results[k]["out"]).astype(np.float32)
        full[a * M1 : (a + 1) * M1, b * M2 : (b + 1) * M2] = blk
    if _trace:
        kernel._last_exec_time_ns = res.exec_time_ns
    return full


# revision 16
# speedup vs baseline: 1.2014x; 1.0118x over previous
"""LSH cosine-of-Hamming retrieval kernel for 8 trn2 NeuronCores.

Math: reference computes cos((pi/d) * hamming(u, v)) for binary LSH codes
u = (emb1 @ r.T > 0), v = (emb2 @ r.T > 0), d = 1024 bits.
With +/-1 sign codes s_u = 2u-1, s_v = 2v-1:
    hamming = (d - s_u . s_v) / 2
    cos((pi/d) * hamming) = cos(pi/2 - (pi/2d) * s_u.s_v) = sin((pi/2d) * s_u.s_v)

Projection: ONE fp16 matmul pass (fp16 has 11 mantissa bits; measured
sign-flip rate 8e-5 -> whole-pipeline rel_err 9.7e-3 vs the 2e-2 gate,
deterministic since inputs/reference are fixed). This is 3x cheaper than
the previous 3-pass bf16 hi/lo scheme.

Binarize: the projection PSUM must be turned into fp8 codes.  Only DVE
(vector) and ACT (scalar) can read PSUM (GPSIMD cannot).  DVE emits
+/-0.5 codes via tensor_scalar(is_gt, subtract 0.5); ACT emits +/-1
codes via the Sign activation.  Bit-chunks are assigned to engines with
complementary parity between u and v, so every bit's code product is
+/-0.5 and the final Sin just uses a 2x scale: out = sin((pi/d) * dot).

Main matmul: fp8 DoubleRow (integer-exact in PSUM f32), 216ns/MM
measured = fp8 peak. Sin activation emits bf16 (adds ~1e-3 rel err,
halves output DMA bytes); host converts back to f32.

Sharding (2x4 grid over 8 cores): core k computes the [2048, 2048] output
block for emb1 rows [(k//4)*2048...] x emb2 rows [(k%4)*2048...]; r is
replicated (collectives measure ~60us fixed cost under this runtime, so
code-sharing across cores does not pay).

Schedule (from ntff trace analysis):
- All input DMAs up front on the two HWDGE queues (sync: first-needed,
  scalar: rest, in consumption order). No gpsimd SWDGE DMAs - their exit
  dge_drain alone cost 4.2us in the baseline.
- v (e2) is projected first since the main loop needs ALL v codes but
  only the m-th row-chunk of u codes; u chunks are interleaved between
  early main m-blocks so binarize throughput (the pacing resource during
  projection: ~0.29us/tile two-engine aggregate vs 0.21us/tile PE) never
  idles the PE for long.
- No warmup matmuls: the framework prologue blocks the PE queue until
  ~7us and input data lands before that, so warmup only delays real work.
"""

import sys

sys.path.insert(0, "/opt/trn_rl_repo")

import ml_dtypes
import numpy as np

import concourse.bacc as bacc
import concourse.tile as tile
from concourse import mybir
from concourse.bass_utils import run_bass_kernel_spmd

N1, N2, D, B = 4096, 8192, 128, 1024  # emb1 rows, emb2 rows, dim, num_bits
G1, G2 = 2, 4
M1, M2 = N1 // G1, N2 // G2  # 2048 x 2048 output block per core
KC = B // 128  # 8 bit-chunks of 128
RW = 512  # projection row-chunk width
NW = 512  # main matmul output tile width

_BUILD_CACHE = {}


def _dedupe_ldweights(nc):
    """Drop back-to-back InstLdweights with identical operands on the PE
    queue. The pipeline emits one weight load per matmul; when consecutive
    matmuls share a stationary operand, the reload is pure overhead. Only
    loads carrying no semaphore waits/updates are removed, so sync
    arithmetic is unchanged; the paired matmuls then use the weights the
    earlier identical load left in the array."""
    removed = 0
    for f in nc.m.functions:
        for bb in f.blocks:
            last_key = None
            for ins in list(bb.instructions):
                if type(ins).__name__ == "InstLdweights":
                    key = ins.concise()
                    if (
                        key == last_key
                        and not ins.has_wait()
                        and not ins.has_update()
                    ):
                        bb.instructions.remove(ins)
                        removed += 1
                    else:
                        last_key = key
    return removed


def _build(scale: float):
    if scale in _BUILD_CACHE:
        return _BUILD_CACHE[scale]
    nc = bacc.Bacc("TRN2", target_bir_lowering=False, debug=False)
    f32 = mybir.dt.float32
    f16 = mybir.dt.float16
    bf16 = mybir.dt.bfloat16
    fp8 = mybir.dt.float8e4

    e1d = nc.declare_dram_parameter("e1", [D, M1], f16, isOutput=False)
    e2d = nc.declare_dram_parameter("e2", [D, M2], f16, isOutput=False)
    rd = nc.declare_dram_parameter("r", [D, B], f16, isOutput=False)
    out = nc.declare_dram_parameter("out", [M1, M2], bf16, isOutput=True)
    junkd = nc.declare_dram_parameter("junk", [128, 8], bf16, isOutput=True)

    with tile.TileContext(nc) as tc:
        with (
            tc.tile_pool(name="const", bufs=1) as const_pool,
            tc.tile_pool(name="codes", bufs=1) as code_pool,
            tc.tile_pool(name="outs", bufs=4) as out_pool,
            tc.tile_pool(name="psum", bufs=4, space="PSUM") as psum_pool,
        ):
            r_sb = const_pool.tile([D, B], f16)
            e1_sb = const_pool.tile([D, M1], f16)
            e2_sb = const_pool.tile([D, M2], f16)

            ut = code_pool.tile([128, KC, M1], fp8)
            vt = code_pool.tile([128, KC, M2], fp8)

            junk_sb = const_pool.tile([128, 8], bf16)

            # Input DMAs up front on the two HWDGE queues, few and fat
            # (wide rows = big descriptors), first-needed pieces first:
            # sync carries the tiny r head + first quarter-chunk of e2 the
            # first matmuls need; scalar streams everything else in
            # consumption order.
            nc.sync.dma_start(r_sb[:, 0:256], rd[:, 0:256])
            nc.sync.dma_start(e2_sb[:, 0:128], e2d[:, 0:128])
            nc.sync.dma_start(e2_sb[:, 128:256], e2d[:, 128:256])
            nc.scalar.dma_start(r_sb[:, 256:], rd[:, 256:])
            nc.scalar.dma_start(e2_sb[:, 256:512], e2d[:, 256:512])
            nc.scalar.dma_start(e2_sb[:, 512:1024], e2d[:, 512:1024])
            nc.scalar.dma_start(e2_sb[:, 1024:2048], e2d[:, 1024:2048])
            nc.scalar.dma_start(e1_sb[:, 0:1024], e1d[:, 0:1024])
            nc.scalar.dma_start(e1_sb[:, 1024:2048], e1d[:, 1024:2048])

            # Warmup: the PE queue clears its prologue ~2.5us before the
            # first input DMA lands; garbage matmuls in that window keep
            # the PE busy so the HAM clock-gate hits 2.4 GHz right as real
            # work starts. The memset runs on gpsimd (idle early, so the
            # matmuls wait on nothing slow); a tiny consumer DMA'd to a
            # junk output keeps DCE from dropping them.
            warm = const_pool.tile([128, RW], f16)
            nc.vector.memset(warm[:], 0.0)
            # 7 cold matmuls ~= 3us: bridges the PE queue from its prologue
            # to the first input DMA's landing (~9.5us) with no PE gap, so
            # the HAM activity window stays unbroken and the clock is at
            # 2.4GHz when real work starts (a gap here resets the window
            # and costs ~6us of half-clock execution).
            wps = psum_pool.tile([128, 2, RW], f32, name="pstile", tag="ps")
            for w in range(7):
                nc.tensor.matmul(
                    wps[:, w % 2, :], warm[:, 0:128], warm[:],
                    start=(w < 2), stop=(w >= 5),
                )
            nc.vector.tensor_scalar(
                junk_sb[:], wps[:, 0, 0:8], 0.0, 0.5,
                mybir.AluOpType.is_gt, mybir.AluOpType.subtract,
            )
            nc.sync.dma_start(junkd[:, :], junk_sb[:])

            def proj_chunk(esb, dst, j, act_parity, w=RW, half=0, c2s=(0, 1, 2, 3)):
                """Project a row-chunk of one side and binarize to fp8
                codes. Bit-chunk-pairs with c2 % 2 == act_parity go to ACT
                (Sign, +/-1 codes), the rest to DVE (is_gt-0.5, +/-0.5).
                u and v use opposite parity so every bit's u*v code
                product is +/-0.5."""
                sl = slice(j * RW + half * w, j * RW + (half + 1) * w)
                for c2 in c2s:
                    ps = psum_pool.tile([128, 2, w], f32, name="pstile", tag="ps")
                    for h in range(2):
                        cs = slice((2 * c2 + h) * 128, (2 * c2 + h + 1) * 128)
                        nc.tensor.matmul(
                            ps[:, h, :], r_sb[:, cs], esb[:, sl],
                            start=True, stop=True,
                        )
                    if c2 % 2 == act_parity:
                        nc.scalar.activation(
                            dst[:, 2 * c2 : 2 * c2 + 2, sl],
                            ps[:],
                            mybir.ActivationFunctionType.Sign,
                        )
                    else:
                        nc.vector.tensor_scalar(
                            dst[:, 2 * c2 : 2 * c2 + 2, sl],
                            ps[:],
                            0.0,
                            0.5,
                            mybir.AluOpType.is_gt,
                            mybir.AluOpType.subtract,
                        )

            def main_block(m):
                """One 128-row output block: fp8 DoubleRow matmul over all
                1024 bits, Sin to bf16, one 512KB output DMA (alternating
                between the two HWDGE queues so neither backs up)."""
                ot = out_pool.tile([128, M2], bf16)
                ms = slice(m * 128, (m + 1) * 128)
                pss = [
                    psum_pool.tile([128, 2, NW], f32, name="pstile", tag="ps")
                    for _ in range(2)
                ]
                for s in range(KC // 2):
                    for t in range(2):
                        for h in range(2):
                            n = 2 * t + h
                            ns = slice(n * NW, (n + 1) * NW)
                            nc.tensor.matmul(
                                pss[t][:, h, :],
                                ut[:, 2 * s : 2 * s + 2, ms],
                                vt[:, 2 * s : 2 * s + 2, ns],
                                start=(s == 0),
                                stop=(s == KC // 2 - 1),
                                perf_mode=mybir.MatmulPerfMode.DoubleRow,
                            )
                for t in range(2):
                    os_ = slice(t * 2 * NW, (t + 1) * 2 * NW)
                    nc.scalar.activation(
                        ot[:, os_],
                        pss[t][:],
                        mybir.ActivationFunctionType.Sin,
                        scale=scale,
                    )
                    # store each half as soon as its Sin lands; sync owns
                    # all output traffic (its 16-channel ring set has
                    # plenty of headroom) so the ACT queue stays clean
                    nc.sync.dma_start(out[ms, os_], ot[:, os_])

            # v first (ACT takes odd bit-pairs), then u chunks (ACT takes
            # even) interleaved with the first main blocks so binarize
            # never stalls the PE: m-block m needs u-chunk m//4 and ALL v.
            # v-chunk 0 runs as four 128-wide quarters so its first
            # matmuls only need the small head DMAs; u-chunks 1-3 are
            # interleaved as bit-halves so each insertion keeps the 4-deep
            # psum ring alternating evenly between projection and main.
            for q in range(4):
                proj_chunk(e2_sb, vt, 0, act_parity=1, w=128, half=q)
            for j in range(1, M2 // RW):
                proj_chunk(e2_sb, vt, j, act_parity=1)
            proj_chunk(e1_sb, ut, 0, act_parity=0)
            main_block(0)
            proj_chunk(e1_sb, ut, 1, act_parity=0, c2s=(0, 1))
            main_block(1)
            proj_chunk(e1_sb, ut, 1, act_parity=0, c2s=(2, 3))
            main_block(2)
            proj_chunk(e1_sb, ut, 2, act_parity=0, c2s=(0, 1))
            main_block(3)
            proj_chunk(e1_sb, ut, 2, act_parity=0, c2s=(2, 3))
            main_block(4)
            proj_chunk(e1_sb, ut, 3, act_parity=0, c2s=(0, 1))
            main_block(5)
            proj_chunk(e1_sb, ut, 3, act_parity=0, c2s=(2, 3))
            for m in range(6, M1 // 128):
                main_block(m)

    # Keep waits on the matmuls (not hoisted to ldweights) so redundant
    # weight loads stay sync-free and can be deduped away.
    nc.move_matmul_waits_to_ldweights = lambda: None
    nc.compile()
    _dedupe_ldweights(nc)
    _BUILD_CACHE[scale] = nc
    return nc


def _in_maps(emb1, emb2, r):
    r16 = np.ascontiguousarray(r.T).astype(np.float16)
    e1t = np.ascontiguousarray(emb1.T).astype(np.float16)
    e2t = np.ascontiguousarray(emb2.T).astype(np.float16)
    maps = []
    for k in range(8):
        a, b = k // G2, k % G2
        s1 = slice(a * M1, (a + 1) * M1)
        s2 = slice(b * M2, (b + 1) * M2)
        maps.append(
            {
                "e1": np.ascontiguousarray(e1t[:, s1]),
                "e2": np.ascontiguousarray(e2t[:, s2]),
                "r": r16,
            }
        )
    return maps


def _install_profile_hook():
    """The agent image's antenv lacks axon_hooks; synthesize it so
    run_bass_kernel_spmd(trace=True) can reach the NTFF profiler."""
    import types

    if "antenv.axon_hooks" in sys.modules:
        return
    try:
        from trn_agent_boot.trn_boot import _ntff_profile_via_ctypes

        hook = _ntff_profile_via_ctypes("/opt/axon/libaxon_pjrt.so")
        mod = types.ModuleType("antenv.axon_hooks")
        mod.get_axon_ntff_profile_hook = lambda: hook
        sys.modules["antenv.axon_hooks"] = mod

        from concourse import bass_utils as _bu

        _orig_upload = _bu.upload_artifacts

        def _safe_upload(tmpdir):
            try:
                return _orig_upload(tmpdir)
            except Exception as e:  # no bucket access in this container
                return f"upload-skipped: {e}"

        _bu.upload_artifacts = _safe_upload
    except Exception:
        pass


def kernel(emb1, emb2, r, pi, _trace=False, _tmpdir=None):
    emb1 = np.asarray(emb1, dtype=np.float32)
    emb2 = np.asarray(emb2, dtype=np.float32)
    r = np.asarray(r, dtype=np.float32)
    # per-bit code product is +/-0.5 -> dot = s_u.s_v / 2, out = sin((pi/d)*dot)
    scale = float(np.asarray(pi).reshape(-1)[0]) / B

    nc = _build(scale)
    if _trace:
        _install_profile_hook()
    try:
        res = run_bass_kernel_spmd(
            nc, _in_maps(emb1, emb2, r), list(range(8)), trace=_trace, tmpdir=_tmpdir
        )
    except ModuleNotFoundError:
        res = run_bass_kernel_spmd(nc, _in_maps(emb1, emb2, r), list(range(8)))

    full = np.empty((N1, N2), dtype=np.float32)
    for k in range(8):
        a, b = k // G2, k % G2
        blk = np.asarray(res.results[k]["out"]).astype(np.float32)
        full[a * M1 : (a + 1) * M1, b * M2 : (b + 1) * M2] = blk
    if _trace:
        kernel._last_exec_time_ns = res.exec_time_ns
    return full
